# revision 1
# baseline (speedup 1.0000x reference)
"""Trainium2 Bass kernel for nn_ByteMulSwiGLU.

Math (per position p of x_bd [B,S,256]):
  mask  = x[0]>0.5 & x[1]>0.5
  a     = first_hot(x[16:32]) + 16*first_hot(x[32:48])      (byte 0..255)
  b     = first_hot(x[48:64]) + 16*first_hot(x[64:80])
  c     = x[107]
  v     = 64-vec with v[0]=a, v[1]=b, v[29]=c  (only row 0 of the 4-row
          x_ge matters: rows are independent and only row 0 col 40 is read)
  y     = swiglu(v, W1_0, W2_0, W3_0)          (64-vec)
  r     = swiglu(y, W1_1, W2_1, W3_1)[40]      (scalar)
  byte  = round(r) mod 256 -> lo/hi nibbles
  out   = x; out[128+lo] += 2*mask; out[144+hi] += 2*mask

Sharding: pure data parallel over batch (8 batches -> 8 cores).

Layer-1 matmuls are exact bf16 (a,b are 8-bit ints = exact bf16; c and the
weights 3-way bf16 split so every product is exact, fp32 PSUM accumulate).
Layer 2 is fused: y only feeds u1/u2, so u1 = (W3_0@W1_1)^T g and
u2c = (W3_0@(W2_1*W3_1[:,40]))^T g with host-precomputed fp64->fp32 products.
r = sum(silu(u1)*u2c) via a PE ones-reduce (G2 as stationary operand).
round() is the 1.5*2^23 magic-number trick (round-half-even fp32).
"""

import os
import numpy as np

try:
    import concourse.bass as bass
except ImportError:
    import sys
    for _p in ("/opt/trn_rl_repo", os.path.expanduser("~/.axon_site/_ro/trn_rl_repo")):
        if os.path.isdir(_p) and _p not in sys.path:
            sys.path.insert(0, _p)
    import concourse.bass as bass

import concourse.mybir as mybir
from concourse import bass_utils
from concourse import bacc
from concourse.tile import TileContext
import ml_dtypes

F32 = mybir.dt.float32
F32R = mybir.dt.float32r
BF16 = mybir.dt.bfloat16
AF = mybir.ActivationFunctionType
OP = mybir.AluOpType

MAGIC = 12582912.0  # 1.5 * 2**23: (x+MAGIC)-MAGIC == round-half-even(x), |x|<2^22

B, S, D = 8, 8192, 256
NCORES = 8


def _bf16_split3(w):
    """Split fp32 array into three bf16 arrays summing exactly to w."""
    w = np.asarray(w, np.float32)
    h = w.astype(ml_dtypes.bfloat16)
    r = w - h.astype(np.float32)
    m = r.astype(ml_dtypes.bfloat16)
    l = (r - m.astype(np.float32)).astype(ml_dtypes.bfloat16)
    return h, m, l


def _wext(W):
    """Layer-1 split weight tile [15, 128] bf16.

    Pairs with CT rows [a,a,a, b,b,b, ch,ch,ch, cm,cm,cm, cl,cl,cl]:
    rows = [w0h,w0m,w0l, w1h,w1m,w1l, (w2h,w2m,w2l)x3] where w*_j are the
    exact 3-way bf16 splits of W rows [0, 1, 29].  One K=15 matmul gives
    a*w0 + b*w1 + (ch+cm+cl)*w2 with every product exact in fp32 PSUM.
    """
    rows = np.asarray(W, np.float32)[[0, 1, 29], :]  # [3,128]
    s0 = _bf16_split3(rows[0])
    s1 = _bf16_split3(rows[1])
    s2 = _bf16_split3(rows[2])
    out = np.zeros((15, 128), dtype=ml_dtypes.bfloat16)
    for j in range(3):
        out[0 + j] = s0[j]
        out[3 + j] = s1[j]
        out[6 + j] = s2[j]
        out[9 + j] = s2[j]
        out[12 + j] = s2[j]
    return out


def make_consts(W1_0, W2_0, W3_0, W1_1, W2_1, W3_1):
    consts = {}
    consts["cWE1"] = _wext(W1_0)
    consts["cWE2"] = _wext(W2_0)
    # Fuse layer-2's first matmul: y is only consumed by u1/u2, so
    # u1 = (W3_0 @ W1_1)^T g and u2c = (W3_0 @ (W2_1 * w3c))^T g.
    # Products computed in fp64, rounded once to fp32.
    w30 = np.asarray(W3_0, np.float64)                         # [128,64]
    w3c = np.asarray(W3_1, np.float64)[:, 40]                  # [128]
    consts["cM1"] = (w30 @ np.asarray(W1_1, np.float64)).astype(np.float32)
    consts["cM2"] = (w30 @ (np.asarray(W2_1, np.float64) * w3c[None, :])
                     ).astype(np.float32)
    rev = (16.0 * (16 - np.arange(16))).astype(np.float32)     # 256,240,...,16
    consts["cREV"] = np.broadcast_to(
        np.tile(rev, 4), (128, 64)).astype(ml_dtypes.bfloat16).copy()
    w4 = np.array([1.0 / 16, 1.0, 1.0 / 16, 1.0], np.float32)
    consts["cW4"] = np.broadcast_to(w4, (128, 4)).astype(ml_dtypes.bfloat16).copy()
    consts["cIOTA"] = np.broadcast_to(
        np.arange(16, dtype=np.float32), (128, 16)).copy()
    consts["cIDEN"] = np.eye(128, dtype=ml_dtypes.bfloat16)
    consts["cONES"] = np.ones((128, 1), np.float32)
    return consts


CONST_SPECS = [
    ("cWE1", [15, 128], BF16), ("cWE2", [15, 128], BF16),
    ("cM1", [128, 128], F32), ("cM2", [128, 128], F32),
    ("cREV", [128, 64], BF16), ("cW4", [128, 4], BF16),
    ("cIOTA", [128, 16], F32), ("cIDEN", [128, 128], BF16),
    ("cONES", [128, 1], F32),
]


def build_nc(groups=4, chunks=16, l2_f32r=False, use_mod=False, stage=99,
             repeat=1, pb=2, ctb=1, xb=3, eqb=3, hb=2, ub=1, rb=1):
    """Build the per-core kernel. S_core = groups*chunks*128 positions."""
    s_core = groups * chunks * 128
    nsub = chunks // 4  # 512-position subtiles per group

    nc = bacc.Bacc(None, target_bir_lowering=False, debug=False)
    x = nc.declare_dram_parameter("x", [s_core, D], F32, isOutput=False)
    out = nc.declare_dram_parameter("out", [s_core, D], F32, isOutput=True)
    # unique per-config param so same-interface variants never collide in
    # the PJRT/NEFF compile caches (they key on the HLO, not the BIR)
    nc.declare_dram_parameter(f"cfg_r{repeat}_s{stage}", [1, 1], F32,
                              isOutput=False)
    mm_dt = F32R if l2_f32r else F32
    R_CONSTS = {"cM1", "cM2"}
    const_specs = [(n, s, (mm_dt if n in R_CONSTS else dt))
                   for n, s, dt in CONST_SPECS]
    cdram = {name: nc.declare_dram_parameter(name, shape, dt, isOutput=False)
             for name, shape, dt in const_specs}

    from contextlib import ExitStack
    with TileContext(nc) as tc, ExitStack() as ctx:
        ep = ctx.enter_context

        cpool = ep(tc.tile_pool(name="const", bufs=1))
        xpool = ep(tc.tile_pool(name="xin", bufs=xb))
        sgpool = ep(tc.tile_pool(name="sg", bufs=2))
        Cpool = ep(tc.tile_pool(name="C", bufs=2))
        expool = ep(tc.tile_pool(name="ex", bufs=2))
        vpool = ep(tc.tile_pool(name="val", bufs=2))
        s2pool = ep(tc.tile_pool(name="s2", bufs=2))
        ctsbp = ep(tc.tile_pool(name="ctsb", bufs=pb))
        g1pool = ep(tc.tile_pool(name="g1", bufs=pb))
        gpool = ep(tc.tile_pool(name="g", bufs=pb))
        s1pool = ep(tc.tile_pool(name="s1", bufs=pb))
        g2pool = ep(tc.tile_pool(name="g2", bufs=pb))
        nibp = ep(tc.tile_pool(name="nib", bufs=2))
        eqpool = ep(tc.tile_pool(name="eq", bufs=eqb))
        # psum pools: ct(ctb) + h(2*hb) + u(2*ub) + r(rb) <= 8 banks
        ctp = ep(tc.tile_pool(name="ctp", bufs=ctb, space="PSUM"))
        hpool = ep(tc.tile_pool(name="h", bufs=hb, space="PSUM"))
        upool = ep(tc.tile_pool(name="u", bufs=ub, space="PSUM"))
        rpool = ep(tc.tile_pool(name="r", bufs=rb, space="PSUM"))

        # --- load constants once ---
        csb = {}
        for name, shape, dt in const_specs:
            t = cpool.tile(shape, dt, tag=name)
            nc.sync.dma_start(t[:], cdram[name][:])
            csb[name] = t
        WE1, WE2 = csb["cWE1"], csb["cWE2"]
        WM1, WM2 = csb["cM1"], csb["cM2"]
        REV, W4, IOTA = csb["cREV"], csb["cW4"], csb["cIOTA"]
        IDEN, ONES = csb["cIDEN"], csb["cONES"]

        REVb = REV[:].rearrange("p (o k) -> p o k", o=1).broadcast_to([128, chunks, 64])
        W4b = W4[:].rearrange("p (o k) -> p o k", o=1).broadcast_to([128, chunks, 4])

        BIASH = cpool.tile([128, 1], F32, tag="biash")
        nc.vector.memset(BIASH[:], -0.5)

        for g in [g for _ in range(repeat) for g in range(groups)]:
            xt = xpool.tile([128, chunks, D], F32, tag="xt")
            xv = x[g * chunks * 128:(g + 1) * chunks * 128, :] \
                .rearrange("(c p) d -> p c d", p=128)
            nc.sync.dma_start(xt[:], xv)

            if stage < 1:
                ov = out[g * chunks * 128:(g + 1) * chunks * 128, :] \
                    .rearrange("(c p) d -> p c d", p=128)
                nc.sync.dma_start(ov, xt[:])
                continue

            # ---- extraction (whole group) ----
            sg = sgpool.tile([128, chunks, 80], BF16, tag="sg")
            nc.scalar.activation(sg[:], xt[:, :, 0:80], AF.Sign, bias=BIASH[:])

            C = Cpool.tile([128, chunks * 32], BF16, tag="C")
            nc.vector.memset(C[:], 0.0)
            Cv = C[:].rearrange("p (c k) -> p c k", k=32)

            val = vpool.tile([128, chunks, 64], BF16, tag="val")
            nc.vector.tensor_tensor(val[:], sg[:, :, 16:80], REVb, OP.mult)

            M = expool.tile([128, chunks, 4], BF16, tag="M")
            nc.vector.tensor_reduce(
                M[:], val[:].rearrange("p c (s j) -> p c s j", j=16),
                axis=mybir.AxisListType.X, op=OP.max)
            M2 = expool.tile([128, chunks, 4], BF16, tag="M2")
            nc.vector.tensor_scalar(M2[:], M[:], 0.0, None, OP.max)
            u = expool.tile([128, chunks, 4], BF16, tag="u")
            nc.vector.tensor_scalar(u[:], M2[:], 0.0, 256.0, OP.is_gt, OP.mult)
            fh = expool.tile([128, chunks, 4], BF16, tag="fh")
            nc.vector.tensor_tensor(fh[:], u[:], M2[:], OP.subtract)
            fhw = expool.tile([128, chunks, 4], BF16, tag="fhw")
            nc.vector.tensor_tensor(fhw[:], fh[:], W4b, OP.mult)
            # bytes -> C cols {0,3} (exact: integer values <= 255)
            with nc.allow_low_precision(reason="byte values <=255 exact in bf16"):
                nc.vector.tensor_reduce(
                    Cv[:, :, 0:6:3], fhw[:].rearrange("p c (b t) -> p c b t", t=2),
                    axis=mybir.AxisListType.X, op=OP.add)
            # op splits -> C cols {6, 9, 12}
            x107 = xt[:, :, 107]
            nc.vector.tensor_copy(Cv[:, :, 6], x107)
            tsp = expool.tile([128, chunks], F32, tag="tsp")
            nc.vector.tensor_tensor(tsp[:], x107, Cv[:, :, 6], OP.subtract)
            nc.vector.tensor_copy(Cv[:, :, 9], tsp[:])
            nc.vector.tensor_tensor(Cv[:, :, 12], tsp[:], Cv[:, :, 9], OP.subtract)
            # replicate each field to 3 adjacent rows: cols {1,4,..13},{2,5,..14}
            nc.vector.tensor_copy(Cv[:, :, 1:16:3], Cv[:, :, 0:15:3])
            nc.vector.tensor_copy(Cv[:, :, 2:17:3], Cv[:, :, 0:15:3])
            # 2*mask
            sab = expool.tile([128, chunks], F32, tag="sab")
            nc.vector.tensor_tensor(sab[:], sg[:, :, 0], sg[:, :, 1], OP.add)
            s2 = s2pool.tile([128, chunks], F32, tag="s2")
            nc.vector.tensor_scalar(s2[:], sab[:], 2.0, 2.0, OP.is_ge, OP.mult)

            for sub in range(nsub if stage >= 2 else 0):
                cbase = sub * 4
                # per-chunk transpose: C[:, 32cc:32cc+15] -> CT[0:15, 128c:+128]
                CT = ctp.tile([15, 512], BF16, tag="ct")
                for c in range(4):
                    cc = cbase + c
                    nc.tensor.transpose(CT[:, 128 * c:128 * (c + 1)],
                                        C[:, 32 * cc:32 * cc + 15], IDEN[:])
                CTsb = ctsbp.tile([15, 512], BF16, tag="ctsb")
                nc.scalar.copy(CTsb[:], CT[:])

                H1 = hpool.tile([128, 512], F32, tag="h1")
                H2 = hpool.tile([128, 512], F32, tag="h2")
                for HT, WE in ((H1, WE1), (H2, WE2)):
                    for c in range(4):
                        nc.tensor.matmul(
                            HT[:, 128 * c:128 * (c + 1)],
                            WE[:], CTsb[:, 128 * c:128 * (c + 1)],
                            start=(c == 0), stop=(c == 3))

                G1 = g1pool.tile([128, 512], F32, tag="g1")
                nc.scalar.activation(G1[:], H1[:], AF.Silu)
                G = gpool.tile([128, 512], mm_dt, tag="g")
                nc.vector.tensor_tensor(G[:], G1[:], H2[:], OP.mult)

                if stage < 3:
                    continue

                U1 = upool.tile([128, 512], F32, tag="u1")
                nc.tensor.matmul(U1[:], WM1[:], G[:])
                U2 = upool.tile([128, 512], F32, tag="u2")
                nc.tensor.matmul(U2[:], WM2[:], G[:])

                S1 = s1pool.tile([128, 512], F32, tag="s1")
                nc.scalar.activation(S1[:], U1[:], AF.Silu)
                G2 = g2pool.tile([128, 512], F32, tag="g2")
                nc.vector.tensor_tensor(G2[:], S1[:], U2[:], OP.mult)

                if stage < 4:
                    continue

                r4 = rpool.tile([128, 4], F32, tag="r4")
                for c in range(4):
                    nc.tensor.matmul(
                        r4[:, c:c + 1],
                        G2[:, 128 * c:128 * (c + 1)], ONES[:],
                        start=True, stop=True)

                # ---- nibble decode (per subtile) ----
                rnd = nibp.tile([128, 4], F32, tag="rnd")
                nc.vector.tensor_scalar(rnd[:], r4[:], MAGIC, -MAGIC, OP.add, OP.add)
                t1 = nibp.tile([128, 4], F32, tag="t1")
                nc.vector.tensor_scalar(t1[:], rnd[:], 1.0 / 256,
                                        -(0.5 - 1.0 / 512), OP.mult, OP.add)
                k = nibp.tile([128, 4], F32, tag="k")
                nc.vector.tensor_scalar(k[:], t1[:], MAGIC, -MAGIC, OP.add, OP.add)
                t2 = nibp.tile([128, 4], F32, tag="t2")
                nc.vector.tensor_scalar(t2[:], k[:], 256.0, None, OP.mult)
                m8 = nibp.tile([128, 4], F32, tag="m8")
                nc.vector.tensor_tensor(m8[:], rnd[:], t2[:], OP.subtract)
                hi = nibp.tile([128, 4], F32, tag="hi")
                nc.vector.tensor_scalar(hi[:], m8[:], 1.0 / 16, -0.46875,
                                        OP.mult, OP.add)
                nc.vector.tensor_scalar(hi[:], hi[:], MAGIC, -MAGIC, OP.add, OP.add)
                lo = nibp.tile([128, 4], F32, tag="lo")
                nc.vector.tensor_scalar(lo[:], hi[:], -16.0, None, OP.mult)
                nc.vector.tensor_tensor(lo[:], m8[:], lo[:], OP.add)

                # ---- delta add + out DMA (per subtile) ----
                for c in range(4):
                    cc = cbase + c
                    eq = eqpool.tile([128, 32], F32, tag="eq")
                    nc.vector.tensor_scalar(eq[:, 0:16], IOTA[:], lo[:, c:c + 1],
                                            s2[:, cc:cc + 1], OP.is_equal, OP.mult)
                    nc.vector.tensor_scalar(eq[:, 16:32], IOTA[:], hi[:, c:c + 1],
                                            s2[:, cc:cc + 1], OP.is_equal, OP.mult)
                    nc.vector.tensor_tensor(xt[:, cc, 128:160], xt[:, cc, 128:160],
                                            eq[:], OP.add)
                ovs = out[(g * chunks + cbase) * 128:(g * chunks + cbase + 4) * 128, :] \
                    .rearrange("(c p) d -> p c d", p=128)
                nc.sync.dma_start(ovs, xt[:, cbase:cbase + 4, :])

            if stage in (1, 2, 3):
                ov = out[g * chunks * 128:(g + 1) * chunks * 128, :] \
                    .rearrange("(c p) d -> p c d", p=128)
                nc.sync.dma_start(ov, xt[:])

    nc.finalize()
    return nc


_NC_CACHE = {}

# test.py can override these before calling kernel()
_TRACE = False
_BUILD_KEY = {}
_LAST_RESULT = None


def _get_nc(key=None):
    kw = dict(_BUILD_KEY if key is None else key)
    hkey = tuple(sorted(kw.items()))
    if hkey not in _NC_CACHE:
        _NC_CACHE[hkey] = build_nc(**kw)
    return _NC_CACHE[hkey]


def kernel(x_bd, W1_0, W2_0, W3_0, W1_1, W2_1, W3_1):
    global _LAST_RESULT
    x_bd = np.ascontiguousarray(np.asarray(x_bd, np.float32))
    consts = make_consts(W1_0, W2_0, W3_0, W1_1, W2_1, W3_1)
    nc = _get_nc()
    in_maps = []
    for core in range(NCORES):
        m = {"x": x_bd[core], "cfg_r1_s99": np.zeros([1, 1], np.float32)}
        m.update(consts)
        in_maps.append(m)
    res = bass_utils.run_bass_kernel_spmd(
        nc, in_maps, core_ids=list(range(NCORES)), trace=_TRACE)
    _LAST_RESULT = res
    out = np.stack([res.results[i]["out"] for i in range(NCORES)], axis=0)
    return out.astype(np.float32)

